# revision 19
# baseline (speedup 1.0000x reference)
"""TRN2 Bass kernel for nn_GCNBasic (2-layer GCN, B=32, N=2048, F=128, H=256).

Sharding: data-parallel over batch B across 8 NeuronCores (4 items/core);
small weights replicated.  A^T is pre-transposed and cast to bf16 on the
HOST, so the device streams it straight into SBUF (8MB/item instead of
16MB) with no GpSimd cast and no PE transposes:

  (AX)^T[f,n]  = sum_mb  X[mb]-stationary   @ A^T[mb]   (rhs 512-wide, bf16)
  H1pre[n,h]   = (AX)^T[:,nb]-stationary    @ W1
  H1           = relu(LN(H1pre + b1))                    (f32 stats, fused)
  (AH)^T[hh,n] = sum_mb H1[mb,hh]-stationary @ A^T[mb]
  H2pre[n,k]   = sum_hh (AH)^T[hh,nb]-stat.  @ diag(g1)W2
  H2           = relu(LN(H2pre + b2))
  g^T          = sum_nb H2[nb,kh]-stationary @ ones  (mean pool via PE)
  outputs      = diag(g2)Wa/Wl heads in fp32, biases added on ACT.

A^T tiles are double-buffered across items (bufs=2) so item i+1's DMA
overlaps item i's compute; the kernel is PE-bound (~46us/item of matmul).

gamma folds (diag(g1)@W2, diag(g2)@Wa/Wl) are exact because relu(g*z)=
g*relu(z) for g>0; beta==0 fast path (the problem's setup_inputs always
produces gamma=1, beta=0); a general gamma/beta path exists as a fallback.

Known TRN2 pitfalls worked around here: tensor_tensor_reduce crashes the
device; ACT/DVE writes into PSUM are unstable -> squares go to SBUF scratch.
"""

from contextlib import ExitStack

import numpy as np
import ml_dtypes

import concourse.bacc as bacc
import concourse.mybir as mybir
import concourse.tile as tile
from concourse.bass_utils import run_bass_kernel_spmd

F32 = mybir.dt.float32
BF16 = mybir.dt.bfloat16
bf16 = ml_dtypes.bfloat16

N = 2048
F = 128
H = 256
K = 64
P = 128
NB = N // P
EPS = 1e-5
N_CORES = 8


def _declare_io(nc, items, general):
    io = {}
    io["a4t"] = nc.dram_tensor("a4t", [items, N, N], BF16, kind="ExternalInput")
    io["x4"] = nc.dram_tensor("x4", [items, P, NB, F], BF16,
                              kind="ExternalInput")
    io["w1"] = nc.dram_tensor("w1", [F, H], BF16, kind="ExternalInput")
    io["w2"] = nc.dram_tensor("w2", [H, H], BF16, kind="ExternalInput")
    io["b1bc"] = nc.dram_tensor("b1bc", [P, H], F32, kind="ExternalInput")
    io["b2bc"] = nc.dram_tensor("b2bc", [P, H], F32, kind="ExternalInput")
    io["wa"] = nc.dram_tensor("wa", [H, K], F32, kind="ExternalInput")
    io["wl"] = nc.dram_tensor("wl", [H, K], F32, kind="ExternalInput")
    io["ba"] = nc.dram_tensor("ba", [K, 1], F32, kind="ExternalInput")
    io["bl"] = nc.dram_tensor("bl", [K, 1], F32, kind="ExternalInput")
    io["ones"] = nc.dram_tensor("ones", [P, 1], BF16, kind="ExternalInput")
    if general:
        io["g1bc"] = nc.dram_tensor("g1bc", [P, H], F32, kind="ExternalInput")
        io["be1bc"] = nc.dram_tensor("be1bc", [P, H], F32, kind="ExternalInput")
        io["g2bc"] = nc.dram_tensor("g2bc", [P, H], F32, kind="ExternalInput")
        io["be2bc"] = nc.dram_tensor("be2bc", [P, H], F32, kind="ExternalInput")
    io["op"] = nc.dram_tensor("op", [items, K], F32, kind="ExternalOutput")
    io["ol"] = nc.dram_tensor("ol", [items, K], F32, kind="ExternalOutput")
    return io


def _build_core(nc, tc, io, items, general, reps=1):
    a4t, x4 = io["a4t"], io["x4"]
    es = ExitStack()

    consts = es.enter_context(tc.tile_pool(name="consts", bufs=1))
    wts = es.enter_context(tc.tile_pool(name="wts", bufs=1))
    pool_at = es.enter_context(tc.tile_pool(name="at", bufs=2))
    pool_xb = es.enter_context(tc.tile_pool(name="xb", bufs=2))
    pool_axT = es.enter_context(tc.tile_pool(name="axT", bufs=2))
    pool_h1 = es.enter_context(tc.tile_pool(name="h1", bufs=2))
    pool_ahT = es.enter_context(tc.tile_pool(name="ahT", bufs=1))
    pool_h2 = es.enter_context(tc.tile_pool(name="h2", bufs=2))
    pool_hc = es.enter_context(tc.tile_pool(name="hc", bufs=2))
    pool_sq = es.enter_context(tc.tile_pool(name="sq", bufs=2))
    pool_st = es.enter_context(tc.tile_pool(name="st", bufs=2))
    pool_gsb = es.enter_context(tc.tile_pool(name="gsb", bufs=4))
    pool_osb = es.enter_context(tc.tile_pool(name="osb", bufs=4))

    ps_big = es.enter_context(tc.tile_pool(name="ps_big", bufs=6, space="PSUM"))
    ps_h = es.enter_context(tc.tile_pool(name="ps_h", bufs=2, space="PSUM"))

    eps_t = consts.tile([P, 1], F32)
    nc.vector.memset(eps_t[:], EPS)
    ones_b = consts.tile([P, 1], BF16)
    nc.sync.dma_start(ones_b[:], io["ones"][:])

    w1_t = wts.tile([P, H], BF16)
    nc.sync.dma_start(w1_t[:], io["w1"][:])
    w2_t = [wts.tile([P, H], BF16, tag=f"w2_{hh}", name=f"w2_{hh}")
            for hh in range(2)]
    for hh in range(2):
        nc.sync.dma_start(w2_t[hh][:], io["w2"][hh * P:(hh + 1) * P, :])
    b1_t = wts.tile([P, H], F32)
    nc.sync.dma_start(b1_t[:], io["b1bc"][:])
    b2_t = wts.tile([P, H], F32)
    nc.sync.dma_start(b2_t[:], io["b2bc"][:])
    wa_t = [wts.tile([P, K], F32, tag=f"wa_{hh}", name=f"wa_{hh}")
            for hh in range(2)]
    wl_t = [wts.tile([P, K], F32, tag=f"wl_{hh}", name=f"wl_{hh}")
            for hh in range(2)]
    for hh in range(2):
        nc.sync.dma_start(wa_t[hh][:], io["wa"][hh * P:(hh + 1) * P, :])
        nc.sync.dma_start(wl_t[hh][:], io["wl"][hh * P:(hh + 1) * P, :])
    ba_t = wts.tile([K, 1], F32)
    nc.sync.dma_start(ba_t[:], io["ba"][:])
    bl_t = wts.tile([K, 1], F32)
    nc.sync.dma_start(bl_t[:], io["bl"][:])
    gb_t = {}
    if general:
        for nm in ("g1bc", "be1bc", "g2bc", "be2bc"):
            t = wts.tile([P, H], F32, tag=nm, name=nm)
            nc.sync.dma_start(t[:], io[nm][:])
            gb_t[nm] = t

    inv_h = 1.0 / H

    import concourse.mybir as _mb

    def ln_stats(nb, ps_pre, b_t, st, hc):
        # One DVE op: hc = ps + b with accum_out = sum(hc); frees the dense
        # psum slot.  One Pool op (otherwise-idle engine): sum of squares.
        nc.vector.scalar_tensor_tensor(
            out=hc[:], in0=ps_pre[:], scalar=1.0, in1=b_t[:],
            op0=mybir.AluOpType.mult, op1=mybir.AluOpType.add,
            accum_out=st[:, 0, nb:nb + 1])
        sq = pool_sq.tile([P, H], BF16, tag="sq", name=f"sq_{nb}")
        nc.scalar.activation(
            out=sq[:], in_=hc[:], func=mybir.ActivationFunctionType.Square,
            accum_out=st[:, 1, nb:nb + 1])

    def finish_stats(st, lo, hi):
        s = st[:, :, lo:hi]
        nc.vector.tensor_scalar(out=s[:, 2, :], in0=s[:, 0, :],
                                scalar1=-inv_h, scalar2=None,
                                op0=mybir.AluOpType.mult)          # -mu
        nc.vector.tensor_tensor(out=s[:, 3, :], in0=s[:, 2, :], in1=s[:, 2, :],
                                op=mybir.AluOpType.mult)           # mu^2
        nc.vector.tensor_scalar(out=s[:, 4, :], in0=s[:, 1, :],
                                scalar1=inv_h, scalar2=None,
                                op0=mybir.AluOpType.mult)          # E[x^2]
        nc.vector.tensor_tensor(out=s[:, 4, :], in0=s[:, 4, :], in1=s[:, 3, :],
                                op=mybir.AluOpType.subtract)       # var
        nc.scalar.activation(out=s[:, 5, :], in_=s[:, 4, :],
                             func=mybir.ActivationFunctionType.Sqrt,
                             bias=eps_t[:], scale=1.0)             # sd
        nc.vector.reciprocal(out=s[:, 6, :], in_=s[:, 5, :])       # 1/sd
        nc.vector.tensor_tensor(out=s[:, 7, :], in0=s[:, 2, :], in1=s[:, 6, :],
                                op=mybir.AluOpType.mult)           # -mu/sd

    def apply_ln(nb, hc, st, h_out, g_bc, be_bc):
        if not general:
            if nb % 2 == 1:
                # DVE path (2 ops) so applies stream on two engines in
                # parallel — they gate the next stage's aggregation.
                nc.vector.tensor_scalar(
                    out=h_out, in0=hc[:], scalar1=st[:, 6, nb:nb + 1],
                    scalar2=st[:, 7, nb:nb + 1],
                    op0=mybir.AluOpType.mult, op1=mybir.AluOpType.add)
                nc.vector.tensor_scalar_max(h_out, h_out, 0.0)
                return
            nc.scalar.activation(out=h_out, in_=hc[:],
                                 func=mybir.ActivationFunctionType.Relu,
                                 bias=st[:, 7, nb:nb + 1],
                                 scale=st[:, 6, nb:nb + 1])
        else:
            nc.scalar.activation(out=hc[:], in_=hc[:],
                                 func=mybir.ActivationFunctionType.Identity,
                                 bias=st[:, 7, nb:nb + 1],
                                 scale=st[:, 6, nb:nb + 1])
            nc.vector.tensor_tensor(out=hc[:], in0=hc[:], in1=g_bc[:],
                                    op=mybir.AluOpType.mult)
            nc.vector.tensor_tensor(out=hc[:], in0=hc[:], in1=be_bc[:],
                                    op=mybir.AluOpType.add)
            nc.scalar.activation(out=h_out, in_=hc[:],
                                 func=mybir.ActivationFunctionType.Relu)

    NCH = N // 512
    state = {}

    def s1(it):
        """Loads + layer-1 aggregation (c-outer) + dense + LN."""
        # xb first (every L1agg matmul needs it), then at tiles in c order,
        # all on the Pool SWDGE queue: it sustains ~340GB/s and delivers
        # tiles in issue order, pacing the c-outer L1agg sweep.  The SP
        # HWDGE queue only manages ~100GB/s here, so it carries just the
        # small weight/output traffic.
        xb = pool_xb.tile([P, NB, F], BF16, tag="xb", name=f"xb_{it}")
        nc.gpsimd.dma_start(xb[:], x4[it])
        at = [pool_at.tile([P, N], BF16, tag=f"at_{c}", name=f"at_{it}_{c}")
              for c in range(NB)]
        for c in range(NB):
            nc.gpsimd.dma_start(at[c][:], a4t[it, c * P:(c + 1) * P, :])

        # (AX)^T: 4 psum chunks live, sweep c outer so at[c] tiles are
        # consumed in DMA-arrival order.
        pb = [ps_big.tile([P, 512], F32, tag="big", name=f"ax_{it}_{j}")
              for j in range(NCH)]
        for c in range(NB):
            for j in range(NCH):
                nc.tensor.matmul(pb[j][:], xb[:, c, :],
                                 at[c][:, j * 512:(j + 1) * 512],
                                 start=(c == 0), stop=(c == NB - 1))
        axT = pool_axT.tile([P, N], BF16, tag="axT", name=f"axT_{it}")
        for j in range(NCH):
            nc.vector.tensor_copy(axT[:, j * 512:(j + 1) * 512], pb[j][:])

        st1 = pool_st.tile([P, 8, NB], F32, tag="st", name=f"st1_{it}")
        h1 = pool_h1.tile([P, NB, H], BF16, tag="h1", name=f"h1_{it}")
        hc1 = []
        for nb in range(NB):
            ph = ps_h.tile([P, H], F32, tag="h", name=f"p1_{it}_{nb}")
            nc.tensor.matmul(ph[:], axT[:, nb * P:(nb + 1) * P], w1_t[:],
                             start=True, stop=True)
            hc = pool_hc.tile([P, H], BF16, tag=f"hc_{nb}",
                              name=f"hc1_{it}_{nb}")
            ln_stats(nb, ph, b1_t, st1, hc)
            hc1.append(hc)
        for q in range(4):
            finish_stats(st1, q * 4, q * 4 + 4)
            for nb in range(q * 4, q * 4 + 4):
                apply_ln(nb, hc1[nb], st1, h1[:, nb, :],
                         gb_t.get("g1bc"), gb_t.get("be1bc"))
        state[it] = {"at": at, "h1": h1}

    def s2a(it):
        """Layer-2 aggregation (hh outer, c outer) + dense + LN."""
        at, h1 = state[it]["at"], state[it]["h1"]
        ahT = [pool_ahT.tile([P, N], BF16, tag=f"ahT_{hh}",
                             name=f"ahT_{it}_{hh}")
               for hh in range(2)]
        for hh in range(2):
            pb = [ps_big.tile([P, 512], F32, tag="big",
                              name=f"ah_{it}_{hh}_{j}")
                  for j in range(NCH)]
            for c in range(NB):
                for j in range(NCH):
                    nc.tensor.matmul(pb[j][:], h1[:, c, hh * P:(hh + 1) * P],
                                     at[c][:, j * 512:(j + 1) * 512],
                                     start=(c == 0), stop=(c == NB - 1))
            for j in range(NCH):
                if j % 2 == 0:
                    nc.scalar.copy(ahT[hh][:, j * 512:(j + 1) * 512], pb[j][:])
                else:
                    nc.vector.tensor_copy(
                        ahT[hh][:, j * 512:(j + 1) * 512], pb[j][:])

        st2 = pool_st.tile([P, 8, NB], F32, tag="st", name=f"st2_{it}")
        h2 = pool_h2.tile([P, NB, H], BF16, tag="h2", name=f"h2_{it}")
        hc2 = []
        for nb in range(NB):
            ph = ps_h.tile([P, H], F32, tag="h", name=f"p2_{it}_{nb}")
            for hh in range(2):
                nc.tensor.matmul(ph[:], ahT[hh][:, nb * P:(nb + 1) * P],
                                 w2_t[hh][:], start=(hh == 0), stop=(hh == 1))
            hc = pool_hc.tile([P, H], BF16, tag=f"hc_{nb}",
                              name=f"hc2_{it}_{nb}")
            ln_stats(nb, ph, b2_t, st2, hc)
            hc2.append(hc)
        for q in range(4):
            finish_stats(st2, q * 4, q * 4 + 4)
            for nb in range(q * 4, q * 4 + 4):
                apply_ln(nb, hc2[nb], st2, h2[:, nb, :],
                         gb_t.get("g2bc"), gb_t.get("be2bc"))
        state[it]["h2"] = h2

    def s2b(it):
        """Mean pool + heads + output DMA."""
        h2 = state[it]["h2"]
        gsb = pool_gsb.tile([P, 2], F32, tag="g", name=f"g_{it}")
        for kh in range(2):
            pg = ps_h.tile([P, H], F32, tag="h", name=f"pg_{it}_{kh}")
            for nb in range(NB):
                nc.tensor.matmul(pg[:, 0:1], h2[:, nb, kh * P:(kh + 1) * P],
                                 ones_b[:], start=(nb == 0),
                                 stop=(nb == NB - 1))
            nc.scalar.mul(gsb[:, kh:kh + 1], pg[:, 0:1], 1.0 / N)

        for hd, (w_t, b_t, out_d) in enumerate(
                ((wa_t, ba_t, io["op"]), (wl_t, bl_t, io["ol"]))):
            po = ps_h.tile([P, H], F32, tag="h", name=f"po_{it}_{hd}")
            for kh in range(2):
                nc.tensor.matmul(po[0:K, 0:1], w_t[kh][:], gsb[:, kh:kh + 1],
                                 start=(kh == 0), stop=(kh == 1))
            osb = pool_osb.tile([K, 1], F32, tag="o", name=f"o_{it}_{hd}")
            nc.scalar.activation(out=osb[:], in_=po[0:K, 0:1],
                                 func=mybir.ActivationFunctionType.Identity,
                                 bias=b_t[:], scale=1.0)
            nc.sync.dma_start(out_d[it:it + 1, :], osb[:])
        del state[it]

    def _body():
        if items == 4:
            # Software pipeline: each item's A^T prefetch and LN latency
            # hides under another item's aggregation matmuls; the deferred
            # s2b(1)/s2b(2) cover the last item's LN latency.
            s1(0); s1(1); s2a(0); s2a(1); s1(2); s2b(0)
            s1(3); s2a(2); s2b(1); s2a(3); s2b(2); s2b(3)
        else:
            for it in range(items):
                s1(it)
                s2a(it)
                s2b(it)

    if reps > 1:
        with tc.For_i(0, reps, 1,
                      hint_engines=(_mb.EngineType.PE, _mb.EngineType.DVE,
                                    _mb.EngineType.Activation,
                                    _mb.EngineType.SP, _mb.EngineType.Pool)):
            _body()
    else:
        _body()

    es.close()


_CACHE = {}


def _get_nc(items, general, reps=1):
    key = (items, general, reps)
    if key not in _CACHE:
        nc = bacc.Bacc("TRN2", target_bir_lowering=False, debug=False,
                       num_devices=N_CORES)
        with tile.TileContext(nc) as tc:
            io = _declare_io(nc, items, general)
            _build_core(nc, tc, io, items, general, reps)
        nc.compile()
        _CACHE[key] = nc
    return _CACHE[key]


def make_in_maps(A_hat, X, W1, b1, g1, beta1, W2, b2, g2, beta2,
                 Wa, ba, Wl, bl):
    """Host-side prep: shard over batch, transpose+cast A, fold gammas."""
    B = A_hat.shape[0]
    items = B // N_CORES
    general = bool(np.any(beta1 != 0) or np.any(beta2 != 0)
                   or np.any(g1 <= 0) or np.any(g2 <= 0))
    if general:
        w2f = np.asarray(W2, np.float32).astype(bf16)
        waf = np.asarray(Wa, np.float32)
        wlf = np.asarray(Wl, np.float32)
    else:
        w2f = (np.asarray(g1, np.float32)[:, None] * W2).astype(bf16)
        waf = (np.asarray(g2, np.float32)[:, None] * Wa).astype(np.float32)
        wlf = (np.asarray(g2, np.float32)[:, None] * Wl).astype(np.float32)
    shared = {
        "w1": np.asarray(W1, np.float32).astype(bf16),
        "w2": w2f,
        "b1bc": np.ascontiguousarray(
            np.broadcast_to(np.asarray(b1, np.float32), (P, H))),
        "b2bc": np.ascontiguousarray(
            np.broadcast_to(np.asarray(b2, np.float32), (P, H))),
        "wa": waf, "wl": wlf,
        "ba": np.asarray(ba, np.float32).reshape(K, 1).copy(),
        "bl": np.asarray(bl, np.float32).reshape(K, 1).copy(),
        "ones": np.ones((P, 1), bf16),
    }
    if general:
        for nm, v in (("g1bc", g1), ("be1bc", beta1),
                      ("g2bc", g2), ("be2bc", beta2)):
            shared[nm] = np.ascontiguousarray(
                np.broadcast_to(np.asarray(v, np.float32), (P, H)))
    A_hat = np.asarray(A_hat, np.float32)
    X = np.asarray(X, np.float32)
    in_maps = []
    for c in range(N_CORES):
        m = dict(shared)
        a = A_hat[c * items:(c + 1) * items]          # [items, n, m]
        m["a4t"] = np.ascontiguousarray(
            a.transpose(0, 2, 1).astype(bf16))        # [items, m, n]
        x = X[c * items:(c + 1) * items]              # [items, (cb p), F]
        m["x4"] = np.ascontiguousarray(
            x.reshape(items, NB, P, F).transpose(0, 2, 1, 3).astype(bf16))
        in_maps.append(m)
    return in_maps, items, general


def kernel(**inputs):
    in_maps, items, general = make_in_maps(**inputs)
    nc = _get_nc(items, general)
    res = run_bass_kernel_spmd(nc, in_maps, core_ids=list(range(N_CORES)))
    pred = np.concatenate([res.results[c]["op"] for c in range(N_CORES)], 0)
    logits = np.concatenate([res.results[c]["ol"] for c in range(N_CORES)], 0)
    return (np.asarray(pred, np.float32), np.asarray(logits, np.float32))


# revision 24
# speedup vs baseline: 1.1397x; 1.1397x over previous
"""TRN2 Bass kernel for nn_GCNBasic (2-layer GCN, B=32, N=2048, F=128, H=256).

Sharding: data-parallel over batch B across 8 NeuronCores (4 items/core);
small weights replicated.  A^T is pre-transposed and cast to bf16 on the
HOST, so the device streams it straight into SBUF (8MB/item instead of
16MB) with no GpSimd cast and no PE transposes:

  (AX)^T[f,n]  = sum_mb  X[mb]-stationary   @ A^T[mb]   (rhs 512-wide, bf16)
  H1pre[n,h]   = (AX)^T[:,nb]-stationary    @ W1
  H1           = relu(LN(H1pre + b1))                    (f32 stats, fused)
  (AH)^T[hh,n] = sum_mb H1[mb,hh]-stationary @ A^T[mb]
  H2pre[n,k]   = sum_hh (AH)^T[hh,nb]-stat.  @ diag(g1)W2
  H2           = relu(LN(H2pre + b2))
  g^T          = sum_nb H2[nb,kh]-stationary @ ones  (mean pool via PE)
  outputs      = diag(g2)Wa/Wl heads in fp32, biases added on ACT.

A^T tiles are double-buffered across items (bufs=2) so item i+1's DMA
overlaps item i's compute; the kernel is PE-bound (~46us/item of matmul).

gamma folds (diag(g1)@W2, diag(g2)@Wa/Wl) are exact because relu(g*z)=
g*relu(z) for g>0; beta==0 fast path (the problem's setup_inputs always
produces gamma=1, beta=0); a general gamma/beta path exists as a fallback.

Known TRN2 pitfalls worked around here: tensor_tensor_reduce crashes the
device; ACT/DVE writes into PSUM are unstable -> squares go to SBUF scratch.
"""

from contextlib import ExitStack

import numpy as np
import ml_dtypes

import concourse.bacc as bacc
import concourse.mybir as mybir
import concourse.tile as tile
from concourse.bass_utils import run_bass_kernel_spmd

F32 = mybir.dt.float32
BF16 = mybir.dt.bfloat16
bf16 = ml_dtypes.bfloat16

N = 2048
F = 128
H = 256
K = 64
P = 128
NB = N // P
EPS = 1e-5
N_CORES = 8


def _declare_io(nc, items, general):
    io = {}
    io["a4t"] = nc.dram_tensor("a4t", [items, N, N], BF16, kind="ExternalInput")
    io["x4"] = nc.dram_tensor("x4", [items, P, NB, F], BF16,
                              kind="ExternalInput")
    io["w1"] = nc.dram_tensor("w1", [F, H], BF16, kind="ExternalInput")
    io["w2"] = nc.dram_tensor("w2", [H, H], BF16, kind="ExternalInput")
    io["b1bc"] = nc.dram_tensor("b1bc", [P, H], F32, kind="ExternalInput")
    io["b2bc"] = nc.dram_tensor("b2bc", [P, H], F32, kind="ExternalInput")
    io["wa"] = nc.dram_tensor("wa", [H, K], F32, kind="ExternalInput")
    io["wl"] = nc.dram_tensor("wl", [H, K], F32, kind="ExternalInput")
    io["ba"] = nc.dram_tensor("ba", [K, 1], F32, kind="ExternalInput")
    io["bl"] = nc.dram_tensor("bl", [K, 1], F32, kind="ExternalInput")
    io["ones"] = nc.dram_tensor("ones", [P, 1], BF16, kind="ExternalInput")
    if general:
        io["g1bc"] = nc.dram_tensor("g1bc", [P, H], F32, kind="ExternalInput")
        io["be1bc"] = nc.dram_tensor("be1bc", [P, H], F32, kind="ExternalInput")
        io["g2bc"] = nc.dram_tensor("g2bc", [P, H], F32, kind="ExternalInput")
        io["be2bc"] = nc.dram_tensor("be2bc", [P, H], F32, kind="ExternalInput")
    io["op"] = nc.dram_tensor("op", [items, K], F32, kind="ExternalOutput")
    io["ol"] = nc.dram_tensor("ol", [items, K], F32, kind="ExternalOutput")
    return io


def _build_core(nc, tc, io, items, general, reps=1):
    a4t, x4 = io["a4t"], io["x4"]
    es = ExitStack()

    consts = es.enter_context(tc.tile_pool(name="consts", bufs=1))
    wts = es.enter_context(tc.tile_pool(name="wts", bufs=1))
    pool_at = es.enter_context(tc.tile_pool(name="at", bufs=2))
    pool_xb = es.enter_context(tc.tile_pool(name="xb", bufs=2))
    pool_axT = es.enter_context(tc.tile_pool(name="axT", bufs=2))
    pool_h1 = es.enter_context(tc.tile_pool(name="h1", bufs=2))
    pool_ahT = es.enter_context(tc.tile_pool(name="ahT", bufs=2))
    pool_h2 = es.enter_context(tc.tile_pool(name="h2", bufs=1))
    pool_hc = es.enter_context(tc.tile_pool(name="hc", bufs=2))
    pool_sq = es.enter_context(tc.tile_pool(name="sq", bufs=2))
    pool_st = es.enter_context(tc.tile_pool(name="st", bufs=2))
    pool_gsb = es.enter_context(tc.tile_pool(name="gsb", bufs=4))
    pool_osb = es.enter_context(tc.tile_pool(name="osb", bufs=4))

    ps_big = es.enter_context(tc.tile_pool(name="ps_big", bufs=6, space="PSUM"))
    ps_h = es.enter_context(tc.tile_pool(name="ps_h", bufs=2, space="PSUM"))

    eps_t = consts.tile([P, 1], F32)
    nc.vector.memset(eps_t[:], EPS)
    ones_b = consts.tile([P, 1], BF16)
    nc.sync.dma_start(ones_b[:], io["ones"][:])

    w1_t = wts.tile([P, H], BF16)
    nc.sync.dma_start(w1_t[:], io["w1"][:])
    w2_t = [wts.tile([P, H], BF16, tag=f"w2_{hh}", name=f"w2_{hh}")
            for hh in range(2)]
    for hh in range(2):
        nc.sync.dma_start(w2_t[hh][:], io["w2"][hh * P:(hh + 1) * P, :])
    b1_t = wts.tile([P, H], F32)
    nc.sync.dma_start(b1_t[:], io["b1bc"][:])
    b2_t = wts.tile([P, H], F32)
    nc.sync.dma_start(b2_t[:], io["b2bc"][:])
    wa_t = [wts.tile([P, K], F32, tag=f"wa_{hh}", name=f"wa_{hh}")
            for hh in range(2)]
    wl_t = [wts.tile([P, K], F32, tag=f"wl_{hh}", name=f"wl_{hh}")
            for hh in range(2)]
    for hh in range(2):
        nc.sync.dma_start(wa_t[hh][:], io["wa"][hh * P:(hh + 1) * P, :])
        nc.sync.dma_start(wl_t[hh][:], io["wl"][hh * P:(hh + 1) * P, :])
    ba_t = wts.tile([K, 1], F32)
    nc.sync.dma_start(ba_t[:], io["ba"][:])
    bl_t = wts.tile([K, 1], F32)
    nc.sync.dma_start(bl_t[:], io["bl"][:])
    gb_t = {}
    if general:
        for nm in ("g1bc", "be1bc", "g2bc", "be2bc"):
            t = wts.tile([P, H], F32, tag=nm, name=nm)
            nc.sync.dma_start(t[:], io[nm][:])
            gb_t[nm] = t

    inv_h = 1.0 / H

    import concourse.mybir as _mb

    def ln_stats(nb, ps_pre, b_t, st, hc):
        # Plain DVE add frees the dense psum slot fast (~270ns); the row
        # sums come from batched per-group reduces (ln_group_reduce) and
        # the squares accumulate on ACT.
        nc.vector.tensor_tensor(out=hc[:, nb, :], in0=ps_pre[:], in1=b_t[:],
                                op=mybir.AluOpType.add)
        sq = pool_sq.tile([P, H], BF16, tag="sq", name=f"sq_{nb}")
        nc.scalar.activation(
            out=sq[:], in_=hc[:, nb, :],
            func=mybir.ActivationFunctionType.Square,
            accum_out=st[:, 1, nb:nb + 1])

    def ln_group_reduce(st, hc, lo, hi):
        # One reduce for a whole group of blocks: [P, k, H] -> [P, k]
        nc.vector.tensor_reduce(out=st[:, 0, lo:hi], in_=hc[:, lo:hi, :],
                                axis=mybir.AxisListType.X,
                                op=mybir.AluOpType.add)

    def finish_stats(st, lo, hi):
        s = st[:, :, lo:hi]
        nc.vector.tensor_scalar(out=s[:, 2, :], in0=s[:, 0, :],
                                scalar1=-inv_h, scalar2=None,
                                op0=mybir.AluOpType.mult)          # -mu
        nc.vector.tensor_tensor(out=s[:, 3, :], in0=s[:, 2, :], in1=s[:, 2, :],
                                op=mybir.AluOpType.mult)           # mu^2
        nc.vector.tensor_scalar(out=s[:, 4, :], in0=s[:, 1, :],
                                scalar1=inv_h, scalar2=None,
                                op0=mybir.AluOpType.mult)          # E[x^2]
        nc.vector.tensor_tensor(out=s[:, 4, :], in0=s[:, 4, :], in1=s[:, 3, :],
                                op=mybir.AluOpType.subtract)       # var
        nc.scalar.activation(out=s[:, 5, :], in_=s[:, 4, :],
                             func=mybir.ActivationFunctionType.Sqrt,
                             bias=eps_t[:], scale=1.0)             # sd
        nc.vector.reciprocal(out=s[:, 6, :], in_=s[:, 5, :])       # 1/sd
        nc.vector.tensor_tensor(out=s[:, 7, :], in0=s[:, 2, :], in1=s[:, 6, :],
                                op=mybir.AluOpType.mult)           # -mu/sd

    def apply_ln(nb, hc, st, h_out, g_bc, be_bc):
        hcs = hc[:, nb, :]
        if not general:
            if nb % 2 == 1:
                # DVE path (2 ops) so applies stream on two engines in
                # parallel — they gate the next stage's aggregation.
                nc.vector.tensor_scalar(
                    out=h_out, in0=hcs, scalar1=st[:, 6, nb:nb + 1],
                    scalar2=st[:, 7, nb:nb + 1],
                    op0=mybir.AluOpType.mult, op1=mybir.AluOpType.add)
                nc.vector.tensor_scalar_max(h_out, h_out, 0.0)
                return
            nc.scalar.activation(out=h_out, in_=hcs,
                                 func=mybir.ActivationFunctionType.Relu,
                                 bias=st[:, 7, nb:nb + 1],
                                 scale=st[:, 6, nb:nb + 1])
        else:
            nc.scalar.activation(out=hcs, in_=hcs,
                                 func=mybir.ActivationFunctionType.Identity,
                                 bias=st[:, 7, nb:nb + 1],
                                 scale=st[:, 6, nb:nb + 1])
            nc.vector.tensor_tensor(out=hcs, in0=hcs, in1=g_bc[:],
                                    op=mybir.AluOpType.mult)
            nc.vector.tensor_tensor(out=hcs, in0=hcs, in1=be_bc[:],
                                    op=mybir.AluOpType.add)
            nc.scalar.activation(out=h_out, in_=hcs,
                                 func=mybir.ActivationFunctionType.Relu)

    def ln_finish_apply(st, hc, h_tile, groups, g_bc, be_bc):
        step = NB // groups
        for g in range(groups):
            lo, hi = g * step, (g + 1) * step
            ln_group_reduce(st, hc, lo, hi)
            finish_stats(st, lo, hi)
            for nb in range(lo, hi):
                apply_ln(nb, hc, st, h_tile[:, nb, :], g_bc, be_bc)

    NCH = N // 512
    state = {}

    def s1(it):
        """Loads + layer-1 aggregation (c-outer) + dense + LN."""
        # xb first (every L1agg matmul needs it), then at tiles in c order,
        # all on the Pool SWDGE queue: it sustains ~340GB/s and delivers
        # tiles in issue order, pacing the c-outer L1agg sweep.  The SP
        # HWDGE queue only manages ~100GB/s here, so it carries just the
        # small weight/output traffic.
        xb = pool_xb.tile([P, NB, F], BF16, tag="xb", name=f"xb_{it}")
        nc.gpsimd.dma_start(xb[:], x4[it])
        at = [pool_at.tile([P, N], BF16, tag=f"at_{c}", name=f"at_{it}_{c}")
              for c in range(NB)]
        for c in range(NB):
            nc.gpsimd.dma_start(at[c][:], a4t[it, c * P:(c + 1) * P, :])

        # (AX)^T: 4 psum chunks live, sweep c outer so at[c] tiles are
        # consumed in DMA-arrival order.
        pb = [ps_big.tile([P, 512], F32, tag="big", name=f"ax_{it}_{j}")
              for j in range(NCH)]
        for c in range(NB):
            for j in range(NCH):
                nc.tensor.matmul(pb[j][:], xb[:, c, :],
                                 at[c][:, j * 512:(j + 1) * 512],
                                 start=(c == 0), stop=(c == NB - 1))
        axT = pool_axT.tile([P, N], BF16, tag="axT", name=f"axT_{it}")
        for j in range(NCH):
            nc.vector.tensor_copy(axT[:, j * 512:(j + 1) * 512], pb[j][:])

        st1 = pool_st.tile([P, 8, NB], F32, tag="st", name=f"st1_{it}")
        h1 = pool_h1.tile([P, NB, H], BF16, tag="h1", name=f"h1_{it}")
        hc1 = pool_hc.tile([P, NB, H], BF16, tag="hc", name=f"hc1_{it}")
        for nb in range(NB):
            ph = ps_h.tile([P, H], F32, tag="h", name=f"p1_{it}_{nb}")
            nc.tensor.matmul(ph[:], axT[:, nb * P:(nb + 1) * P], w1_t[:],
                             start=True, stop=True)
            ln_stats(nb, ph, b1_t, st1, hc1)
        ln_finish_apply(st1, hc1, h1, 2, gb_t.get("g1bc"), gb_t.get("be1bc"))
        state[it] = {"at": at, "h1": h1}

    def s2a(it):
        """Layer-2 aggregation (hh outer, c outer) + dense + LN."""
        at, h1 = state[it]["at"], state[it]["h1"]
        ahT = [pool_ahT.tile([P, N], BF16, tag=f"ahT_{hh}",
                             name=f"ahT_{it}_{hh}")
               for hh in range(2)]
        for hh in range(2):
            pb = [ps_big.tile([P, 512], F32, tag="big",
                              name=f"ah_{it}_{hh}_{j}")
                  for j in range(NCH)]
            for c in range(NB):
                for j in range(NCH):
                    nc.tensor.matmul(pb[j][:], h1[:, c, hh * P:(hh + 1) * P],
                                     at[c][:, j * 512:(j + 1) * 512],
                                     start=(c == 0), stop=(c == NB - 1))
            for j in range(NCH):
                if j % 2 == 0:
                    nc.scalar.copy(ahT[hh][:, j * 512:(j + 1) * 512], pb[j][:])
                else:
                    nc.vector.tensor_copy(
                        ahT[hh][:, j * 512:(j + 1) * 512], pb[j][:])

        st2 = pool_st.tile([P, 8, NB], F32, tag="st", name=f"st2_{it}")
        h2 = pool_h2.tile([P, NB, H], BF16, tag="h2", name=f"h2_{it}")
        hc2 = pool_hc.tile([P, NB, H], BF16, tag="hc", name=f"hc2_{it}")
        for nb in range(NB):
            ph = ps_h.tile([P, H], F32, tag="h", name=f"p2_{it}_{nb}")
            for hh in range(2):
                nc.tensor.matmul(ph[:], ahT[hh][:, nb * P:(nb + 1) * P],
                                 w2_t[hh][:], start=(hh == 0), stop=(hh == 1))
            ln_stats(nb, ph, b2_t, st2, hc2)
        groups = 4 if it == items - 1 else 2
        ln_finish_apply(st2, hc2, h2, groups,
                        gb_t.get("g2bc"), gb_t.get("be2bc"))
        state[it]["h2"] = h2

    def s2b(it):
        """Mean pool + heads + output DMA."""
        h2 = state[it]["h2"]
        gsb = pool_gsb.tile([P, 2], F32, tag="g", name=f"g_{it}")
        for kh in range(2):
            pg = ps_h.tile([P, H], F32, tag="h", name=f"pg_{it}_{kh}")
            for nb in range(NB):
                nc.tensor.matmul(pg[:, 0:1], h2[:, nb, kh * P:(kh + 1) * P],
                                 ones_b[:], start=(nb == 0),
                                 stop=(nb == NB - 1))
            nc.scalar.mul(gsb[:, kh:kh + 1], pg[:, 0:1], 1.0 / N)

        for hd, (w_t, b_t, out_d) in enumerate(
                ((wa_t, ba_t, io["op"]), (wl_t, bl_t, io["ol"]))):
            po = ps_h.tile([P, H], F32, tag="h", name=f"po_{it}_{hd}")
            for kh in range(2):
                nc.tensor.matmul(po[0:K, 0:1], w_t[kh][:], gsb[:, kh:kh + 1],
                                 start=(kh == 0), stop=(kh == 1))
            osb = pool_osb.tile([K, 1], F32, tag="o", name=f"o_{it}_{hd}")
            nc.scalar.activation(out=osb[:], in_=po[0:K, 0:1],
                                 func=mybir.ActivationFunctionType.Identity,
                                 bias=b_t[:], scale=1.0)
            nc.sync.dma_start(out_d[it:it + 1, :], osb[:])
        del state[it]

    def _body():
        if items == 4:
            # Software pipeline: each item's A^T prefetch and LN latency
            # hides under another item's aggregation matmuls; the deferred
            # s2b(1)/s2b(2) cover the last item's LN latency.
            s1(0); s1(1); s2a(0); s2a(1); s1(2); s2b(0)
            s1(3); s2a(2); s2b(1); s2a(3); s2b(2); s2b(3)
        else:
            for it in range(items):
                s1(it)
                s2a(it)
                s2b(it)

    if reps > 1:
        with tc.For_i(0, reps, 1,
                      hint_engines=(_mb.EngineType.PE, _mb.EngineType.DVE,
                                    _mb.EngineType.Activation,
                                    _mb.EngineType.SP, _mb.EngineType.Pool)):
            _body()
    else:
        _body()

    es.close()


_CACHE = {}


def _get_nc(items, general, reps=1):
    key = (items, general, reps)
    if key not in _CACHE:
        nc = bacc.Bacc("TRN2", target_bir_lowering=False, debug=False,
                       num_devices=N_CORES)
        with tile.TileContext(nc) as tc:
            io = _declare_io(nc, items, general)
            _build_core(nc, tc, io, items, general, reps)
        nc.compile()
        _CACHE[key] = nc
    return _CACHE[key]


def make_in_maps(A_hat, X, W1, b1, g1, beta1, W2, b2, g2, beta2,
                 Wa, ba, Wl, bl):
    """Host-side prep: shard over batch, transpose+cast A, fold gammas."""
    B = A_hat.shape[0]
    items = B // N_CORES
    general = bool(np.any(beta1 != 0) or np.any(beta2 != 0)
                   or np.any(g1 <= 0) or np.any(g2 <= 0))
    if general:
        w2f = np.asarray(W2, np.float32).astype(bf16)
        waf = np.asarray(Wa, np.float32)
        wlf = np.asarray(Wl, np.float32)
    else:
        w2f = (np.asarray(g1, np.float32)[:, None] * W2).astype(bf16)
        waf = (np.asarray(g2, np.float32)[:, None] * Wa).astype(np.float32)
        wlf = (np.asarray(g2, np.float32)[:, None] * Wl).astype(np.float32)
    shared = {
        "w1": np.asarray(W1, np.float32).astype(bf16),
        "w2": w2f,
        "b1bc": np.ascontiguousarray(
            np.broadcast_to(np.asarray(b1, np.float32), (P, H))),
        "b2bc": np.ascontiguousarray(
            np.broadcast_to(np.asarray(b2, np.float32), (P, H))),
        "wa": waf, "wl": wlf,
        "ba": np.asarray(ba, np.float32).reshape(K, 1).copy(),
        "bl": np.asarray(bl, np.float32).reshape(K, 1).copy(),
        "ones": np.ones((P, 1), bf16),
    }
    if general:
        for nm, v in (("g1bc", g1), ("be1bc", beta1),
                      ("g2bc", g2), ("be2bc", beta2)):
            shared[nm] = np.ascontiguousarray(
                np.broadcast_to(np.asarray(v, np.float32), (P, H)))
    A_hat = np.asarray(A_hat, np.float32)
    X = np.asarray(X, np.float32)
    in_maps = []
    for c in range(N_CORES):
        m = dict(shared)
        a = A_hat[c * items:(c + 1) * items]          # [items, n, m]
        m["a4t"] = np.ascontiguousarray(
            a.transpose(0, 2, 1).astype(bf16))        # [items, m, n]
        x = X[c * items:(c + 1) * items]              # [items, (cb p), F]
        m["x4"] = np.ascontiguousarray(
            x.reshape(items, NB, P, F).transpose(0, 2, 1, 3).astype(bf16))
        in_maps.append(m)
    return in_maps, items, general


def kernel(**inputs):
    in_maps, items, general = make_in_maps(**inputs)
    nc = _get_nc(items, general)
    res = run_bass_kernel_spmd(nc, in_maps, core_ids=list(range(N_CORES)))
    pred = np.concatenate([res.results[c]["op"] for c in range(N_CORES)], 0)
    logits = np.concatenate([res.results[c]["ol"] for c in range(N_CORES)], 0)
    return (np.asarray(pred, np.float32), np.asarray(logits, np.float32))
